# revision 1
# baseline (speedup 1.0000x reference)
"""Trainium2 Bass kernel for BandSplitModule (masked LN per band + weight-normed Linear).

Strategy:
  - Data-parallel over T (2048 = 8 cores x 256). No collectives.
  - Host folds weight-norm + LN affine into a single per-band weight matrix
    W2[n] = (g * v / ||v||) * (gamma * mask) with a bias row
    bias2[n] = W @ (beta * mask) + bias, prepended as contraction row 0
    (the device appends a ones column to xhat so the matmul adds the bias).
  - Features permuted from (c, k, reim) to (k, c, reim) order so each band's
    features are one contiguous slice of a [t=128, (F+64)*4] SBUF slab.
  - Runtime band_start/band_width are baked into the compiled program
    (compilation happens inside kernel(); results cached per band structure).
  - Device per band-tile: bn_stats/bn_aggr -> rsqrt -> tensor_scalar xhat,
    TensorE transpose -> matmul (k = 4w+1 chunks) -> z[E, T] psum -> out.
"""
import os
import numpy as np

B, C, F, T, E = 4, 2, 1025, 2048, 128
MAX_BW = 65
NB = 37
EPS = 1e-5
NCORES = 8
TLOC = T // NCORES  # 256
FPAD = F + MAX_BW - 1  # 1089
D = C * MAX_BW * 2  # 260

LAST_EXEC_NS = None

_PLAN_CACHE = {}


def _ensure_trace_hook():
    """Install the antenv.axon_hooks NTFF-profile shim (missing on this image)
    so run_bass_kernel_spmd(trace=True) can capture HW exec time. Fully
    optional — any failure leaves the plain execution path untouched."""
    try:
        import sys, types

        if "antenv.axon_hooks" not in sys.modules:
            mod = types.ModuleType("antenv.axon_hooks")
            _h = {"hook": None}
            mod.set_axon_ntff_profile_hook = lambda h: _h.__setitem__("hook", h)
            mod.get_axon_ntff_profile_hook = lambda: _h["hook"]
            sys.modules["antenv.axon_hooks"] = mod
            try:
                import antenv

                antenv.axon_hooks = mod
            except Exception:
                pass
            try:
                from trn_agent_boot.trn_boot import _ntff_profile_via_ctypes

                hook = _ntff_profile_via_ctypes("/opt/axon/libaxon_pjrt.so")
                if hook is not None:
                    mod.set_axon_ntff_profile_hook(hook)
            except Exception:
                pass
        import concourse.bass_utils as bu

        if not getattr(bu, "_offline_upload_patch", False):
            bu.upload_artifacts = lambda tmpdir: tmpdir
            bu._offline_upload_patch = True
    except Exception:
        pass


def _feature_perm():
    # new index (k,c,r) -> reference index (c,k,r)
    kk, cc, rr = np.meshgrid(
        np.arange(MAX_BW), np.arange(C), np.arange(2), indexing="ij"
    )
    new_i = (kk * 4 + cc * 2 + rr).reshape(-1)
    src_i = (cc * (MAX_BW * 2) + kk * 2 + rr).reshape(-1)
    perm = np.empty(D, np.int64)
    perm[new_i] = src_i
    return perm


def _fold_weights(ln_gamma, ln_beta, v, g, bias, widths):
    karr = np.arange(MAX_BW)
    bw_mask = karr[None, :] < widths[:, None]
    fm = (
        np.broadcast_to(bw_mask[:, None, :, None], (NB, C, MAX_BW, 2))
        .reshape(NB, D)
        .astype(np.float32)
    )
    vnorm = np.sqrt((v * v).sum(-1, keepdims=True))
    W = g[..., None] * v / vnorm
    W2 = W * (ln_gamma * fm)[:, None, :]
    bias2 = np.einsum("ned,nd->ne", W, ln_beta * fm) + bias
    W2p = W2[:, :, _feature_perm()]  # [NB, E, D] in (k,c,r) order
    return W2p, bias2


def _pack_weights(W2p, widths):
    """Pack per-band [k_n = 4w, E] weight rows into SBUF-layout chunks of 128."""
    kns = np.maximum(4 * widths, 4).astype(np.int64)
    nchunks = np.maximum(1, (kns + 127) // 128).astype(np.int64)
    tot_chunks = int(nchunks.sum())
    Wt = np.zeros((128, tot_chunks * 128), np.float32)
    chunk_base = np.zeros(NB, np.int64)
    cb = 0
    for n in range(NB):
        chunk_base[n] = cb
        kn = int(kns[n])
        w4 = 4 * int(widths[n])
        col = np.zeros((kn, E), np.float32)
        if w4 > 0:
            col[:w4] = W2p[n, :, :w4].T
        for j in range(int(nchunks[n])):
            cs = min(128, kn - j * 128)
            Wt[:cs, (cb + j) * 128 : (cb + j) * 128 + E] = col[j * 128 : j * 128 + cs]
        cb += int(nchunks[n])
    return Wt, kns, nchunks, chunk_base, tot_chunks


def _prep_x(x):
    """x [B,C,F,T,2] f32 -> x4 [NCORES, B, TLOC, FPAD*4] with (k,c,r) features, padded."""
    xr = np.transpose(x, (0, 3, 2, 1, 4)).reshape(B, T, F, 4)  # [B,T,F,(c,r)]
    x4 = np.empty((B, T, FPAD, 4), np.float32)
    x4[:, :, :F, :] = xr
    x4[:, :, F:, :] = xr[:, :, F - 1 : F, :]
    x4 = x4.reshape(B, NCORES, TLOC, FPAD * 4)
    x4 = np.ascontiguousarray(np.transpose(x4, (1, 0, 2, 3)))
    return x4  # [NCORES, B, TLOC, FPAD*4]


def _build_program(kns, nchunks, chunk_base, tot_chunks, starts):
    import concourse.bass as bass
    import concourse.bacc as bacc
    import concourse.tile as tile
    from concourse import mybir
    from concourse.masks import make_identity
    from contextlib import ExitStack

    f32 = mybir.dt.float32
    bf16 = mybir.dt.bfloat16
    nc = bacc.Bacc()
    x_ext = nc.declare_dram_parameter("x4", [B, TLOC, FPAD * 4], bf16, isOutput=False)
    wt_ext = nc.declare_dram_parameter(
        "wt", [128, tot_chunks * 128], bf16, isOutput=False
    )
    bias_ext = nc.declare_dram_parameter("bias2", [E, NB], f32, isOutput=False)
    z_ext = nc.declare_dram_parameter("out", [NB, B, E, TLOC], bf16, isOutput=True)

    with ExitStack() as ctx:
        tc = ctx.enter_context(tile.TileContext(nc))
        consts = ctx.enter_context(tc.tile_pool(name="consts", bufs=1))
        slabs = ctx.enter_context(tc.tile_pool(name="slabs", bufs=1))
        stats = ctx.enter_context(tc.tile_pool(name="stats", bufs=24))
        xh_pool = ctx.enter_context(tc.tile_pool(name="xh", bufs=12))
        xt_pool = ctx.enter_context(tc.tile_pool(name="xt", bufs=16))
        zs_pool = ctx.enter_context(tc.tile_pool(name="zs", bufs=8))
        tp_psum = ctx.enter_context(tc.tile_pool(name="tp", bufs=4, space="PSUM"))
        z_psum = ctx.enter_context(tc.tile_pool(name="zp", bufs=4, space="PSUM"))

        ident = consts.tile([128, 128], bf16)
        make_identity(nc, ident)
        eps_t = consts.tile([128, 1], f32)
        nc.vector.memset(eps_t, EPS)
        wt_sb = consts.tile([128, tot_chunks * 128], bf16)
        nc.sync.dma_start(out=wt_sb, in_=wt_ext[:, :])
        bias_sb = consts.tile([E, NB], f32)
        nc.sync.dma_start(out=bias_sb, in_=bias_ext[:, :])

        slab_tiles = {}
        for b in range(B):
            for t0 in range(TLOC // 128):
                st = slabs.tile([128, FPAD * 4], bf16, tag=f"slab_{b}_{t0}")
                nc.sync.dma_start(
                    out=st, in_=x_ext[b, t0 * 128 : (t0 + 1) * 128, :]
                )
                slab_tiles[(b, t0)] = st

        NSUB = B * (TLOC // 128)  # 8 stat subtiles per band
        for n in range(NB):
            kn = int(kns[n])
            s4 = 4 * int(starts[n])
            nch = int(nchunks[n])
            cb = int(chunk_base[n])
            # --- stats for all 8 subtiles of this band, batched scalars ---
            mvb = stats.tile([128, 2 * NSUB], f32, tag="mvb")
            for i, (b, t0) in enumerate(
                (b, t0) for b in range(B) for t0 in range(TLOC // 128)
            ):
                xsl = slab_tiles[(b, t0)][:, s4 : s4 + kn]
                stt = stats.tile([128, 6], f32)
                nc.vector.bn_stats(out=stt, in_=xsl)
                nc.vector.bn_aggr(out=mvb[:, 2 * i : 2 * i + 2], in_=stt)
            vrb = stats.tile([128, NSUB], f32, tag="vrb")
            nc.vector.tensor_copy(out=vrb, in_=mvb[:, 1 : 2 * NSUB : 2])
            rsb = stats.tile([128, NSUB], f32, tag="rsb")
            nc.scalar.activation(
                out=rsb,
                in_=vrb,
                func=mybir.ActivationFunctionType.Sqrt,
                bias=eps_t,
                scale=1.0,
            )
            nc.vector.reciprocal(out=rsb, in_=rsb)
            mrb = stats.tile([128, NSUB], f32, tag="mrb")
            nc.vector.tensor_mul(mrb, mvb[:, 0 : 2 * NSUB : 2], rsb)
            negmr = stats.tile([128, NSUB], f32, tag="negmr")
            nc.scalar.mul(out=negmr, in_=mrb, mul=-1.0)
            mvh = mvb
            rsh = rsb
            for b in range(B):
                xhs = []
                for t0 in range(TLOC // 128):
                    i = b * (TLOC // 128) + t0
                    xsl = slab_tiles[(b, t0)][:, s4 : s4 + kn]
                    xh_t = xh_pool.tile([128, 260], bf16)
                    if i % 2 == 0:
                        nc.vector.tensor_scalar(
                            out=xh_t[:, :kn],
                            in0=xsl,
                            scalar1=mvh[:, 2 * i : 2 * i + 1],
                            scalar2=rsh[:, i : i + 1],
                            op0=mybir.AluOpType.subtract,
                            op1=mybir.AluOpType.mult,
                        )
                    else:
                        nc.scalar.activation(
                            out=xh_t[:, :kn],
                            in_=xsl,
                            func=mybir.ActivationFunctionType.Identity,
                            scale=rsh[:, i : i + 1],
                            bias=negmr[:, i : i + 1],
                        )
                    xhs.append(xh_t)
                zp = z_psum.tile([128, 256], f32)
                for j in range(nch):
                    cs = min(128, kn - j * 128)
                    tp = tp_psum.tile([128, 256], bf16)
                    for t0 in range(2):
                        nc.tensor.transpose(
                            out=tp[:cs, t0 * 128 : (t0 + 1) * 128],
                            in_=xhs[t0][:, j * 128 : j * 128 + cs],
                            identity=ident,
                        )
                    xt = xt_pool.tile([128, 256], bf16)
                    nc.any.tensor_copy(out=xt[:cs, :], in_=tp[:cs, :])
                    nc.tensor.matmul(
                        zp,
                        lhsT=wt_sb[:cs, (cb + j) * 128 : (cb + j) * 128 + E],
                        rhs=xt[:cs, :],
                        start=(j == 0),
                        stop=(j == nch - 1),
                    )
                zs = zs_pool.tile([128, 256], bf16)
                nc.scalar.activation(
                    out=zs,
                    in_=zp,
                    func=mybir.ActivationFunctionType.Identity,
                    bias=bias_sb[:, n : n + 1],
                    scale=1.0,
                )
                nc.sync.dma_start(out=z_ext[n, b, :, :], in_=zs)
    nc.compile()
    return nc


def kernel(x, ln_gamma, ln_beta, v, g, bias, band_start, band_width):
    global LAST_EXEC_NS
    _ensure_trace_hook()
    from concourse.bass_utils import run_bass_kernel_spmd

    x = np.asarray(x, np.float32)
    ln_gamma = np.asarray(ln_gamma, np.float32)
    ln_beta = np.asarray(ln_beta, np.float32)
    v = np.asarray(v, np.float32)
    g = np.asarray(g, np.float32)
    bias = np.asarray(bias, np.float32)
    starts = np.asarray(band_start).astype(np.int64)
    widths = np.asarray(band_width).astype(np.int64)

    import ml_dtypes

    W2p, bias2 = _fold_weights(ln_gamma, ln_beta, v, g, bias, widths)
    Wt, kns, nchunks, chunk_base, tot_chunks = _pack_weights(W2p, widths)
    x4 = _prep_x(x)

    bf = ml_dtypes.bfloat16
    x4b = x4.astype(bf)
    Wtb = Wt.astype(bf)
    bias2t = np.ascontiguousarray(bias2.T)  # [E, NB] f32

    key = (tuple(starts.tolist()), tuple(widths.tolist()))
    if key not in _PLAN_CACHE:
        _PLAN_CACHE[key] = _build_program(
            kns, nchunks, chunk_base, tot_chunks, starts
        )
    nc = _PLAN_CACHE[key]

    in_maps = [
        {"x4": x4b[i], "wt": Wtb, "bias2": bias2t} for i in range(NCORES)
    ]
    res = run_bass_kernel_spmd(nc, in_maps, core_ids=list(range(NCORES)))
    LAST_EXEC_NS = res.exec_time_ns

    zarr = np.stack([np.asarray(r["out"]) for r in res.results]).astype(
        np.float32
    )  # [8, NB, B, E, TLOC]
    z = np.transpose(zarr, (2, 1, 0, 4, 3)).reshape(B, NB, T, E)
    return np.ascontiguousarray(z)



# revision 8
# speedup vs baseline: 1.2289x; 1.2289x over previous
"""Trainium2 Bass kernel for BandSplitModule (masked LN per band + weight-normed Linear).

Strategy (v2 — LN folded into the matmul, K-major everything):
  - Data-parallel over T (2048 = 8 cores x 256). No collectives.
  - Host folds weight-norm + LN affine into W2 = (g*v/||v||)*(gamma*mask),
    bias2 = W@(beta*mask) + bias, then CENTERS the weights:
    W' = W2 - outer(rowsum(W2)/n, mask) so that W'@x = W2@(x - mean).
    Device computes z = rs * (W' @ x) + bias2 with rs = 1/sqrt(var+eps).
  - x is shipped K-major: [33 chunks, 128 k, B*TLOC] bf16. The main matmul
    uses x chunks as the PE stationary (out [t, E] per band), so the final
    scale by rs[t] is a per-partition scalar op.
  - Per-band stats are batched on the TensorEngine: mask/n-stationary
    matmuls stream x and x*x, accumulating Sx=[37,1024], Sxx=[37,1024]
    in PSUM across all 33 chunks. var = Sxx - Sx^2, rs = 1/sqrt(var+eps),
    then 8 tiny PE transposes give rs in [t,37] layout.
  - Output drains (psum->sbuf, *rs) round-robin across Scalar/Vector/GpSimd;
    bias2 is added with one whole-slab [128, 37*128] vector add per slab.
  - Runtime band_start/band_width are baked into the compiled program
    (compilation happens inside kernel(); results cached per band structure).
"""
import numpy as np

B, C, F, T, E = 4, 2, 1025, 2048, 128
MAX_BW = 65
NB = 37
EPS = 1e-5
NCORES = 8
TLOC = T // NCORES  # 256
D = C * MAX_BW * 2  # 260
KF = 4 * F  # 4100 global features (freq-major: k = 4f + 2c + r)
NC = 33  # k chunks of 128
K = NC * 128  # 4224 padded
NSLAB = B * (TLOC // 128)  # 8 output slabs per core
ZW = NB * E  # 4736 output slab width

LAST_EXEC_NS = None

_PLAN_CACHE = {}


def _ensure_trace_hook():
    """Install the antenv.axon_hooks NTFF-profile shim (missing on this image)
    so run_bass_kernel_spmd(trace=True) can capture HW exec time. Fully
    optional — any failure leaves the plain execution path untouched."""
    try:
        import sys, types

        if "antenv.axon_hooks" not in sys.modules:
            mod = types.ModuleType("antenv.axon_hooks")
            _h = {"hook": None}
            mod.set_axon_ntff_profile_hook = lambda h: _h.__setitem__("hook", h)
            mod.get_axon_ntff_profile_hook = lambda: _h["hook"]
            sys.modules["antenv.axon_hooks"] = mod
            try:
                import antenv

                antenv.axon_hooks = mod
            except Exception:
                pass
            try:
                from trn_agent_boot.trn_boot import _ntff_profile_via_ctypes

                hook = _ntff_profile_via_ctypes("/opt/axon/libaxon_pjrt.so")
                if hook is not None:
                    mod.set_axon_ntff_profile_hook(hook)
            except Exception:
                pass
        import concourse.bass_utils as bu

        if not getattr(bu, "_offline_upload_patch", False):
            bu.upload_artifacts = lambda tmpdir: tmpdir
            bu._offline_upload_patch = True
    except Exception:
        pass


def _feature_perm():
    # new index (k,c,r) -> reference index (c,k,r), within one band
    kk, cc, rr = np.meshgrid(
        np.arange(MAX_BW), np.arange(C), np.arange(2), indexing="ij"
    )
    new_i = (kk * 4 + cc * 2 + rr).reshape(-1)
    src_i = (cc * (MAX_BW * 2) + kk * 2 + rr).reshape(-1)
    perm = np.empty(D, np.int64)
    perm[new_i] = src_i
    return perm


def _band_rows(starts, widths):
    """Per band: the global k rows its (clipped) features map to.
    Returns list of arrays rows[n] of length 4*w_n (duplicates where the
    reference's freq clip at F-1 folds several kk onto the same row)."""
    rows = []
    for s, w in zip(starts, widths):
        kk = np.arange(int(w))
        f = np.clip(int(s) + kk, 0, F - 1)
        r4 = (4 * f[:, None] + np.arange(4)[None, :]).reshape(-1)  # (k-major)
        rows.append(r4)
    return rows


def _fold_weights(ln_gamma, ln_beta, v, g, bias, starts, widths):
    """Returns Wg [K, NB*E] f32 (centered, k-major global rows), bias2 [NB,E],
    maskn [K, NB] f32 (1/n per valid row)."""
    karr = np.arange(MAX_BW)
    bw_mask = karr[None, :] < widths[:, None]
    fm = (
        np.broadcast_to(bw_mask[:, None, :, None], (NB, C, MAX_BW, 2))
        .reshape(NB, D)
        .astype(np.float32)
    )
    vnorm = np.sqrt((v * v).sum(-1, keepdims=True))
    vnorm = np.where(vnorm == 0, 1.0, vnorm)
    W = g[..., None] * v / vnorm
    W2 = W * (ln_gamma * fm)[:, None, :]
    bias2 = np.einsum("ned,nd->ne", W, ln_beta * fm) + bias
    W2p = W2[:, :, _feature_perm()]  # [NB, E, D] in (k,c,r) order

    rows = _band_rows(starts, widths)
    Wg = np.zeros((K, NB * E), np.float32)
    maskn = np.zeros((K, NB), np.float32)
    for n in range(NB):
        w = int(widths[n])
        if w == 0:
            continue
        nfeat = float(4 * w)
        wsum2 = W2p[n, :, : 4 * w].sum(axis=1)  # [E]
        Wc = W2p[n, :, : 4 * w].T - wsum2[None, :] / nfeat  # [4w, E] centered
        np.add.at(Wg, (rows[n], slice(n * E, (n + 1) * E)), Wc)
        np.add.at(maskn, (rows[n], n), 1.0 / nfeat)
    return Wg, bias2, maskn


def _plan_chunks(starts, widths):
    """Per 128-row chunk: matmul groups [(wcol, ncols, bands, start, stop)].
    Bands fully inside a chunk are merged pairwise (psum tile is 256 wide)."""
    ranges = []
    for s, w in zip(starts, widths):
        lo = 4 * min(int(s), F - 1)
        hi = 4 * min(int(s) + int(w), F)
        ranges.append((lo, hi))
    chunk_groups = []
    wcol = 0
    for c in range(NC):
        clo, chi = 128 * c, 128 * c + 128
        groups = []
        run = []  # accumulating mergeable full bands
        for n in range(NB):
            lo, hi = ranges[n]
            if hi <= lo or hi <= clo or lo >= chi:
                continue
            full = lo >= clo and hi <= chi
            if full and len(run) < 2:
                run.append(n)
                continue
            if run:
                groups.append((wcol, 128 * len(run), tuple(run), True, True))
                wcol += 128 * len(run)
                run = []
            if full:
                run.append(n)
            else:
                groups.append((wcol, 128, (n,), lo >= clo, hi <= chi))
                wcol += 128
        if run:
            groups.append((wcol, 128 * len(run), tuple(run), True, True))
            wcol += 128 * len(run)
        chunk_groups.append(groups)
    return chunk_groups, wcol


def _pack_wt(Wg, chunk_groups, wcols):
    Wt = np.zeros((128, wcols), np.float32)
    for c, groups in enumerate(chunk_groups):
        sl = Wg[128 * c : 128 * c + 128]
        for wcol, ncols, bands, _, _ in groups:
            off = wcol
            for n in bands:
                Wt[:, off : off + E] = sl[:, n * E : (n + 1) * E]
                off += E
    return Wt


def _prep_x(x):
    """x [B,C,F,T,2] f32 -> [NCORES, NC, 128, B*TLOC] bf16 (k-major chunks)."""
    import ml_dtypes

    xr = np.ascontiguousarray(x.transpose(2, 1, 4, 0, 3)).reshape(KF, B, T)
    xk = np.zeros((K, B, T), ml_dtypes.bfloat16)
    xk[:KF] = xr
    # cols per core: b-major (b*TLOC + t)
    xk = xk.reshape(K, B, NCORES, TLOC).transpose(2, 0, 1, 3)
    xk = np.ascontiguousarray(xk.reshape(NCORES, NC, 128, B * TLOC))
    return xk


def _build_program(chunk_groups, wcols, widths):
    import concourse.bass as bass
    import concourse.bacc as bacc
    import concourse.tile as tile
    from concourse import mybir
    from concourse.masks import make_identity
    from contextlib import ExitStack

    f32 = mybir.dt.float32
    bf16 = mybir.dt.bfloat16
    TC = 1024  # B * TLOC columns per core
    nc = bacc.Bacc()
    x_ext = nc.declare_dram_parameter("xk", [NC, 128, TC], bf16, isOutput=False)
    wt_ext = nc.declare_dram_parameter("wt", [128, wcols], bf16, isOutput=False)
    mk_ext = nc.declare_dram_parameter("mk", [128, NC * NB], bf16, isOutput=False)
    b2_ext = nc.declare_dram_parameter("b2", [128, ZW], bf16, isOutput=False)
    z_ext = nc.declare_dram_parameter("out", [NSLAB, 128, ZW], bf16, isOutput=True)

    with ExitStack() as ctx:
        tc = ctx.enter_context(tile.TileContext(nc))
        consts = ctx.enter_context(tc.tile_pool(name="consts", bufs=1))
        xpool = ctx.enter_context(tc.tile_pool(name="x", bufs=1))
        x2pool = ctx.enter_context(tc.tile_pool(name="x2", bufs=4))
        rspool = ctx.enter_context(tc.tile_pool(name="rs", bufs=1))
        zspool = ctx.enter_context(tc.tile_pool(name="zs", bufs=3))
        st_psum = ctx.enter_context(tc.tile_pool(name="st", bufs=1, space="PSUM"))
        z_psum = ctx.enter_context(tc.tile_pool(name="zp", bufs=3, space="PSUM"))
        rs_psum = ctx.enter_context(tc.tile_pool(name="rp", bufs=1, space="PSUM"))

        ident = consts.tile([128, 128], f32)
        make_identity(nc, ident)
        eps_t = consts.tile([128, 1], f32)
        nc.vector.memset(eps_t, EPS)
        wt_sb = consts.tile([128, wcols], bf16)
        nc.sync.dma_start(out=wt_sb, in_=wt_ext[:, :])
        mk_sb = consts.tile([128, NC * NB], bf16)
        nc.sync.dma_start(out=mk_sb, in_=mk_ext[:, :])
        b2_sb = consts.tile([128, ZW], bf16)
        nc.sync.dma_start(out=b2_sb, in_=b2_ext[:, :])

        xt = []
        for c in range(NC):
            t = xpool.tile([128, TC], bf16, tag=f"x{c}")
            nc.sync.dma_start(out=t, in_=x_ext[c])
            xt.append(t)

        # ---- Phase 1: batched stats on PE (Sx = sum x/n, Sxx = sum x^2/n) ----
        Sx0 = st_psum.tile([128, 512], f32, tag="Sx0")
        Sx1 = st_psum.tile([128, 512], f32, tag="Sx1")
        Sxx0 = st_psum.tile([128, 512], f32, tag="Sxx0")
        Sxx1 = st_psum.tile([128, 512], f32, tag="Sxx1")
        for c in range(NC):
            x2 = x2pool.tile([128, TC], bf16)
            nc.vector.tensor_mul(x2, xt[c], xt[c])
            mk_c = mk_sb[:, c * NB : c * NB + NB]
            st = c == 0
            sp = c == NC - 1
            nc.tensor.matmul(Sx0[:NB, :], lhsT=mk_c, rhs=xt[c][:, 0:512], start=st, stop=sp)
            nc.tensor.matmul(Sx1[:NB, :], lhsT=mk_c, rhs=xt[c][:, 512:1024], start=st, stop=sp)
            nc.tensor.matmul(Sxx0[:NB, :], lhsT=mk_c, rhs=x2[:, 0:512], start=st, stop=sp)
            nc.tensor.matmul(Sxx1[:NB, :], lhsT=mk_c, rhs=x2[:, 512:1024], start=st, stop=sp)

        # ---- Phase 2: rs = 1/sqrt(Sxx - Sx^2 + eps)  [NB, 1024] f32 ----
        var_t = rspool.tile([128, TC], f32, tag="var")
        sqr_t = rspool.tile([128, TC], f32, tag="sqr")
        rs_t = rspool.tile([128, TC], f32, tag="rsv")
        for h, (Sx, Sxx) in enumerate(((Sx0, Sxx0), (Sx1, Sxx1))):
            sl = slice(512 * h, 512 * h + 512)
            nc.scalar.square(out=var_t[:NB, sl], in_=Sx[:NB, :])
            nc.vector.tensor_sub(var_t[:NB, sl], Sxx[:NB, :], var_t[:NB, sl])
        nc.scalar.activation(
            out=sqr_t[:NB, :],
            in_=var_t[:NB, :],
            func=mybir.ActivationFunctionType.Sqrt,
            bias=eps_t[:NB, :],
            scale=1.0,
        )
        nc.vector.reciprocal(out=rs_t[:NB, :], in_=sqr_t[:NB, :])

        # ---- Phase 3: transpose rs to [t, NB] per slab ----
        rsT = []
        for s in range(NSLAB):
            rp = rs_psum.tile([128, NB], f32, tag="rp")
            nc.tensor.transpose(
                out=rp, in_=rs_t[:NB, 128 * s : 128 * s + 128], identity=ident[:NB, :NB]
            )
            rt = rspool.tile([128, NB], f32, tag=f"rsT{s}")
            nc.vector.tensor_copy(out=rt, in_=rp)
            rsT.append(rt)

        # ---- Phase 4: per-slab matmul sweep + drains + bias + store ----
        drain_rr = 0
        for s in range(NSLAB):
            toff = 128 * s
            zslab = zspool.tile([128, ZW], bf16, tag="zslab")
            live = {}
            for c in range(NC):
                x_c = xt[c][:, toff : toff + 128]
                for wcol, ncols, bands, st, sp in chunk_groups[c]:
                    key = bands[0]
                    if st:
                        live[key] = z_psum.tile([128, 256], f32, tag="zp", name="zp")
                    zp = live[key]
                    nc.tensor.matmul(
                        zp[:, :ncols],
                        lhsT=x_c,
                        rhs=wt_sb[:, wcol : wcol + ncols],
                        start=st,
                        stop=sp,
                    )
                    if sp:
                        for j, n in enumerate(bands):
                            dst = zslab[:, n * E : (n + 1) * E]
                            src = zp[:, j * E : j * E + E]
                            sc = rsT[s][:, n : n + 1]
                            r = drain_rr % 5
                            drain_rr += 1
                            if r in (0, 2, 4):
                                nc.scalar.mul(out=dst, in_=src, mul=sc)
                            else:
                                nc.vector.tensor_scalar(
                                    dst, src, sc, None, mybir.AluOpType.mult
                                )
                        del live[key]
            nc.vector.tensor_add(zslab, zslab, b2_sb)
            nc.sync.dma_start(out=z_ext[s], in_=zslab)
    nc.compile()
    return nc


def kernel(x, ln_gamma, ln_beta, v, g, bias, band_start, band_width):
    global LAST_EXEC_NS
    _ensure_trace_hook()
    from concourse.bass_utils import run_bass_kernel_spmd
    import ml_dtypes

    x = np.asarray(x, np.float32)
    ln_gamma = np.asarray(ln_gamma, np.float32)
    ln_beta = np.asarray(ln_beta, np.float32)
    v = np.asarray(v, np.float32)
    g = np.asarray(g, np.float32)
    bias = np.asarray(bias, np.float32)
    starts = np.asarray(band_start).astype(np.int64)
    widths = np.asarray(band_width).astype(np.int64)

    bf = ml_dtypes.bfloat16
    Wg, bias2, maskn = _fold_weights(ln_gamma, ln_beta, v, g, bias, starts, widths)
    chunk_groups, wcols = _plan_chunks(starts, widths)
    Wt = _pack_wt(Wg, chunk_groups, wcols).astype(bf)
    mk = np.ascontiguousarray(
        maskn.reshape(NC, 128, NB).transpose(1, 0, 2).reshape(128, NC * NB)
    ).astype(bf)
    b2 = np.broadcast_to(bias2.reshape(1, ZW), (128, ZW)).astype(bf)
    xk = _prep_x(x)

    key = (tuple(starts.tolist()), tuple(widths.tolist()))
    if key not in _PLAN_CACHE:
        _PLAN_CACHE[key] = _build_program(chunk_groups, wcols, widths)
    nc = _PLAN_CACHE[key]

    in_maps = [{"xk": xk[i], "wt": Wt, "mk": mk, "b2": b2} for i in range(NCORES)]
    res = run_bass_kernel_spmd(nc, in_maps, core_ids=list(range(NCORES)))
    LAST_EXEC_NS = res.exec_time_ns

    zarr = np.stack([np.asarray(r["out"]) for r in res.results]).astype(np.float32)
    # [NCORES, NSLAB(b,t2), 128, NB*E] -> [B, NB, T, E]
    z = zarr.reshape(NCORES, B, 2, 128, NB, E)
    z = z.transpose(1, 4, 0, 2, 3, 5).reshape(B, NB, T, E)
    return np.ascontiguousarray(z)
